# revision 22
# baseline (speedup 1.0000x reference)
"""Trainium2 Bass kernel for nn_DeepSeekNeuralMLP (SwiGLU MLP with
Catmull-Rom-spline-reconstructed weights), tensor-parallel over 8 NeuronCores.

Strategy (Megatron-style, fused single-phase):
  - gate/up weights [8192, 2048] sharded over the intermediate dim: core r owns
    rows [r*1024, (r+1)*1024).  down weight [2048, 8192] sharded over its input
    (intermediate) dim: core r owns columns [r*1024, (r+1)*1024).  Each core
    produces a partial output [2048, 8192] in bf16; the host sums the 8
    partials in f32 and transposes to the final [4, 2048, 2048].
  - Spline reconstruction runs on-device into bf16 weight tiles held in SBUF
    for the whole kernel.  The static sampling grid factors into 128-sample
    chunks; within a chunk the control interval index takes at most two values
    (j_c, j_c+1), so each chunk is two cubic evaluations blended by a static
    mask.  Host-side input prep gathers control-point taps and scales them by
    static u-powers (z-rows); the device does two small matmuls per 512-chunk
    block (static Vandermonde-style lhsT VA/VB) and a copy + predicated-patch
    select.
  - Single fused main loop over 16 blocks of 512 tokens: gate/up matmuls ->
    SwiGLU -> down matmul straight out of SBUF (no DRAM spill of the
    intermediate).
  - HAM discipline: K=21 gen matmuls only light one array quadrant and count
    as PE-idle for the HAM clock gate, so a gen phase run in isolation sits at
    the cold 1.2 GHz clock.  Fix: full-K dummy matmuls warm the gate at t=0,
    and all weight generation (beyond the first two super-blocks) is WOVEN
    between the full-K matmuls of token-block 0, which keeps the activity
    monitor busy and hides the gen drains (scalar copy + predicated patch)
    under main-loop PE work.  gate/up use the strided chunk order so
    it-iteration i depends only on super-blocks {2i, 2i+1}.
"""
import contextlib
import numpy as np
import ml_dtypes
from math import comb

import concourse.bass as bass
from concourse import bacc, tile, mybir
from concourse.bass_utils import run_bass_kernel_spmd

# ----------------------------------------------------------------------------
# static problem geometry (hardcoded; must match the reference)
# ----------------------------------------------------------------------------
HIDDEN = 2048
INTER = 8192
NTOK = 8192                    # 4 * 2048 tokens
NCORES = 8
N = INTER * HIDDEN             # samples per weight (same for all three)
NCTRL = max(16, int(N / 128.9))
NCHUNK = N // 128
CPB = NCHUNK // NCORES         # 16384 chunks per core per weight
IC = INTER // NCORES           # 1024 intermediate per core

F32 = mybir.dt.float32
F32R = mybir.dt.float32r
BF16 = mybir.dt.bfloat16
U8 = mybir.dt.uint8

_B_COEF = 0.5 * np.array([
    [0.0, -1.0,  2.0, -1.0],
    [2.0,  0.0, -5.0,  3.0],
    [0.0,  1.0,  4.0, -3.0],
    [0.0,  0.0, -1.0,  1.0],
], dtype=np.float64)           # Catmull-Rom basis b_t(f) coeffs, [tap, power]


def _static_tables():
    t = np.linspace(0.0, NCTRL - 1.0, N, dtype=np.float64)
    i = np.clip(np.floor(t).astype(np.int64), 0, NCTRL - 2)
    k0 = np.arange(NCHUNK, dtype=np.int64) * 128
    j = i[k0]
    iv = i.reshape(NCHUNK, 128)
    m = (iv == j[:, None]).sum(axis=1)
    u = t[k0] - j
    delta = (NCTRL - 1.0) / (N - 1.0)
    return j, u, m, delta


_J, _U, _M, _DELTA = _static_tables()


def _bderiv(y):
    y = np.asarray(y, dtype=np.float64)
    out = np.zeros((4, 4) + y.shape, dtype=np.float64)
    for e in range(4):
        for tp in range(4):
            for p in range(e, 4):
                out[e, tp] += comb(p, e) * _B_COEF[tp, p] * y ** (p - e)
    return out


def _va_vb():
    """Row map: z = (e-1)*5 + tau for e in 1..3 (u-power-scaled tap rows),
    z = 15 dummy zero row, z = 16 + tau for raw (e = 0) tap rows."""
    s = np.arange(128, dtype=np.float64)
    dA = _bderiv(s * _DELTA)
    dB = _bderiv(s * _DELTA - 1.0)
    VA = np.zeros((21, 128), dtype=np.float64)
    VB = np.zeros((21, 128), dtype=np.float64)
    for e in range(4):
        for tp in range(4):
            zA = 16 + tp if e == 0 else (e - 1) * 5 + tp
            zB = 16 + (tp + 1) if e == 0 else (e - 1) * 5 + (tp + 1)
            VA[zA] = dA[e, tp]
            VB[zB] = dB[e, tp]
    return VA.astype(np.float32), VB.astype(np.float32)


_VA, _VB = _va_vb()


def _chunklists():
    """gate/up (strided order): tile column q = i_local*16 + hb; lhsT tile
    (it, kt) = cols [it*2048 + kt : +2033 : 16]; tile (it, *) only touches
    cols [it*2048, (it+1)*2048) = super-blocks {2it, 2it+1}.
    down (contiguous order): tile column q = ib*2048 + h <-> global chunk
    h*64 + r*8 + ib; lhsT tile (ht, it) = cols [it*2048 + ht*128, +128)."""
    gu = np.arange(NCHUNK, dtype=np.int64).reshape(NCORES, CPB)
    h = np.arange(HIDDEN, dtype=np.int64)
    ib = np.arange(8, dtype=np.int64)
    dn = np.empty((NCORES, CPB), dtype=np.int64)
    for r in range(NCORES):
        dn[r] = (h[None, :] * 64 + r * 8 + ib[:, None]).reshape(-1)
    return gu, dn


_CL_GU, _CL_DN = _chunklists()


def _static_for_clist(cl):
    """cp gather indices [5, CPB], u-power rows [15, CPB], mask [128, CPB]."""
    j = _J[cl]
    u = _U[cl]
    m = _M[cl]
    idx = np.clip(j[None, :] + np.arange(-1, 4)[:, None], 0, NCTRL - 1)
    us = np.zeros((15, cl.size), dtype=np.float32)
    for e in range(1, 4):
        us[(e - 1) * 5:(e - 1) * 5 + 5, :] = (u ** e).astype(np.float32)[None, :]
    s = np.arange(128, dtype=np.int64)
    mask = (s[:, None] >= m[None, :]).astype(np.uint8)
    return idx, np.ascontiguousarray(us), np.ascontiguousarray(mask)


_STATIC_GU = [_static_for_clist(_CL_GU[r]) for r in range(NCORES)]
_STATIC_DN = [_static_for_clist(_CL_DN[r]) for r in range(NCORES)]


def _zp_for(cp, idx, us):
    """z-operand [21, CPB]: rows 0..14 = taps * u^e (e=1..3), row 15 = 0,
    rows 16..20 = raw taps."""
    rows = np.take(cp, idx).astype(np.float32)
    zp = np.zeros((21, idx.shape[1]), dtype=np.float32)
    zp[0:15] = np.tile(rows, (3, 1)) * us
    zp[16:21] = rows
    return np.ascontiguousarray(zp)


def _zp4_for(cp, idx, us):
    """Row-packed z-operand [128, CPB/4] for tile_position generation: four
    consecutive 512-chunk blocks stacked on partition strips 32i+z.  Quad qb
    covers blocks 4qb..4qb+3; strip i holds block 4qb+i."""
    zp = _zp_for(cp, idx, us)
    zpv = zp.reshape(21, CPB // 2048, 4, 512)     # [z, qb, i, t]
    zp4 = np.zeros((128, CPB // 4), dtype=np.float32)
    for i in range(4):
        zp4[32 * i:32 * i + 21] = zpv[:, :, i, :].reshape(21, CPB // 4)
    return np.ascontiguousarray(zp4)


def _va4_vb4():
    va4 = np.zeros((128, 128), dtype=np.float32)
    vb4 = np.zeros((128, 128), dtype=np.float32)
    for i in range(4):
        va4[32 * i:32 * i + 21] = _VA
        vb4[32 * i:32 * i + 21] = _VB
    return va4, vb4


_VA4, _VB4 = _va4_vb4()


# ----------------------------------------------------------------------------
# device program
# ----------------------------------------------------------------------------
def _build_program():
    nc = bacc.Bacc("TRN2", target_bir_lowering=False, debug=False,
                   num_devices=NCORES)

    hsT = nc.dram_tensor("hsT", [HIDDEN, NTOK], BF16, kind="ExternalInput")
    dum_d = nc.dram_tensor("dum", [128, 512], BF16, kind="ExternalInput")
    va_d = nc.dram_tensor("va", [128, 128], F32, kind="ExternalInput")
    vb_d = nc.dram_tensor("vb", [128, 128], F32, kind="ExternalInput")
    zp_d = {w: nc.dram_tensor(f"zp_{w}", [128, CPB // 4], F32,
                              kind="ExternalInput")
            for w in ("gate", "up", "down")}
    mask_gu_d = nc.dram_tensor("mask_gu", [128, CPB], U8, kind="ExternalInput")
    mask_dn_d = nc.dram_tensor("mask_dn", [128, CPB], U8, kind="ExternalInput")
    outT = nc.dram_tensor("outT", [HIDDEN, NTOK], BF16, kind="ExternalOutput")

    with tile.TileContext(nc) as tc:
        with contextlib.ExitStack() as ctx:
            pools = {
                "const": ctx.enter_context(tc.tile_pool(name="const", bufs=1)),
                "w": ctx.enter_context(tc.tile_pool(name="w", bufs=1)),
                "zp": ctx.enter_context(tc.tile_pool(name="zp", bufs=8)),
                "mask": ctx.enter_context(tc.tile_pool(name="mask", bufs=8)),
                "hs": ctx.enter_context(tc.tile_pool(name="hs", bufs=32)),
                "sil": ctx.enter_context(tc.tile_pool(name="sil", bufs=3)),
                "inter": ctx.enter_context(tc.tile_pool(name="inter", bufs=12)),
                "out": ctx.enter_context(tc.tile_pool(name="out", bufs=4)),
                "psum": ctx.enter_context(
                    tc.tile_pool(name="psum", bufs=4, space="PSUM")),
                "gpsum": ctx.enter_context(
                    tc.tile_pool(name="gpsum", bufs=4, space="PSUM")),
            }
            dum_t = pools["const"].tile([128, 512], BF16, tag="dum")
            nc.sync.dma_start(dum_t[:], dum_d[:])
            va_t = pools["const"].tile([128, 128], F32, tag="va")
            vb_t = pools["const"].tile([128, 128], F32, tag="vb")
            nc.sync.dma_start(va_t[:], va_d[:])
            nc.sync.dma_start(vb_t[:], vb_d[:])

            gate_t = pools["w"].tile([128, CPB], BF16, tag="gate", name="gate_t")
            up_t = pools["w"].tile([128, CPB], BF16, tag="up", name="up_t")
            down_t = pools["w"].tile([128, CPB], BF16, tag="down", name="down_t")

            # ---- weight generation: tile_position row-packed quads --------
            # Quad qb of weight w = blocks 4qb..4qb+3 (weight cols
            # [qb*2048,(qb+1)*2048)).  The four K=32 strip matmuls run
            # concurrently on distinct array row-groups, so the full 128-row
            # array is active — unlike K=21 matmuls, this counts as PE-busy
            # for the HAM clock gate.
            wmap = {}
            gen_stash = {}

            def quad_A(w, qb, pa_pool):
                wtile, zdram, mdram = wmap[w]
                zq = pools["zp"].tile([128, 512], F32, tag="zp", name="zq")
                nc.sync.dma_start(zq[:], zdram[:, qb * 512:(qb + 1) * 512])
                pas, mts = [], []
                for i in range(4):
                    pa = pools[pa_pool].tile([128, 512], F32, tag="ps",
                                             name="pa4")
                    nc.tensor.matmul(pa[:], va_t[32 * i:32 * i + 32, :],
                                     zq[32 * i:32 * i + 32, :],
                                     start=True, stop=True,
                                     tile_position=(32 * i, 0))
                    pas.append(pa)
                for i in range(4):
                    col = (4 * qb + i) * 512
                    mt = pools["mask"].tile([128, 512], U8, tag="mask",
                                            name="mt")
                    nc.sync.dma_start(mt[:], mdram[:, col:col + 512])
                    nc.scalar.copy(wtile[:, col:col + 512], pas[i][:])
                    mts.append(mt)
                gen_stash[(w, qb)] = (zq, mts)

            def quad_B(w, qb, pb_pool):
                wtile, zdram, mdram = wmap[w]
                zq, mts = gen_stash.pop((w, qb))
                pbs = []
                for i in range(4):
                    pb = pools[pb_pool].tile([128, 512], F32, tag="ps",
                                             name="pb4")
                    nc.tensor.matmul(pb[:], vb_t[32 * i:32 * i + 32, :],
                                     zq[32 * i:32 * i + 32, :],
                                     start=True, stop=True,
                                     tile_position=(32 * i, 0))
                    pbs.append(pb)
                for i in range(4):
                    col = (4 * qb + i) * 512
                    nc.vector.copy_predicated(wtile[:, col:col + 512],
                                              mts[i][:], pbs[i][:])

            def emit_hs(tb, kts=range(16)):
                tiles = []
                for kt in kts:
                    t = pools["hs"].tile([128, 512], BF16, tag="hs", name="hst")
                    nc.sync.dma_start(
                        t[:], hsT[kt * 128:(kt + 1) * 128, tb * 512:(tb + 1) * 512])
                    tiles.append(t)
                return tiles

            def emit_it_iter(it, hs_tiles, hooks=None):
                """One intermediate block: 32 gate/up matmuls + SwiGLU.
                `hooks` maps kt -> callables emitted after that kt's matmuls
                (used to weave down-weight generation under main PE work)."""
                pg = pools["psum"].tile([128, 512], F32, tag="ps", name="pg")
                pu = pools["psum"].tile([128, 512], F32, tag="ps", name="pu")
                for kt in range(16):
                    base = it * 2048 + kt
                    lg = gate_t[:, base:base + 2033:16]
                    lu = up_t[:, base:base + 2033:16]
                    rhs = hs_tiles[kt][:]
                    nc.tensor.matmul(pg[:], lg, rhs,
                                     start=(kt == 0), stop=(kt == 15))
                    nc.tensor.matmul(pu[:], lu, rhs,
                                     start=(kt == 0), stop=(kt == 15))
                    if hooks and kt in hooks:
                        for fn in hooks[kt]:
                            fn()
                sil = pools["sil"].tile([128, 512], F32, tag="sil", name="sil")
                nc.scalar.activation(sil[:], pg[:],
                                     mybir.ActivationFunctionType.Silu)
                itile = pools["inter"].tile([128, 512], BF16, tag="itile",
                                            name="itile")
                nc.vector.tensor_mul(itile[:], sil[:], pu[:])
                return itile

            def emit_ht_loop(tb, int_tiles):
                for ht in range(16):
                    pd = pools["psum"].tile([128, 512], F32, tag="ps", name="pd")
                    for it in range(8):
                        base = it * 2048 + ht * 128
                        ld = down_t[:, base:base + 128]
                        nc.tensor.matmul(pd[:], ld, int_tiles[it][:],
                                         start=(it == 0), stop=(it == 7))
                    ot = pools["out"].tile([128, 512], BF16, tag="ot", name="ot")
                    nc.scalar.copy(ot[:], pd[:])
                    nc.sync.dma_start(
                        outT[ht * 128:(ht + 1) * 128, tb * 512:(tb + 1) * 512],
                        ot[:])

            wmap.update({
                "gate": (gate_t, zp_d["gate"], mask_gu_d),
                "up": (up_t, zp_d["up"], mask_gu_d),
                "down": (down_t, zp_d["down"], mask_dn_d),
            })

            # ---- schedule ----
            # HAM warm-up: full-K dummy matmuls (never read) flip the clock
            # gate while the first gen DMAs land.
            for _ in range(16):
                pdum = pools["gpsum"].tile([128, 512], F32, tag="ps",
                                           name="pdum")
                nc.tensor.matmul(pdum[:], dum_t[:, 0:128], dum_t[:],
                                 start=True, stop=True)

            # gate/up generation, serial, drain-bound (~44us): A-matmuls use
            # the gen PSUM pool, B-matmuls the main pool (idle until tb0).
            for qb in range(8):
                quad_A("gate", qb, "gpsum")
                quad_B("gate", qb, "psum")
                quad_A("up", qb, "gpsum")
                quad_B("up", qb, "psum")

            hs_cur = emit_hs(0)

            # re-warm right before the dense stream (the drain-bound gen
            # phase above has low PE duty and lets the clock gate re-throttle)
            for _ in range(8):
                pdum = pools["gpsum"].tile([128, 512], F32, tag="ps",
                                           name="pdum2")
                nc.tensor.matmul(pdum[:], dum_t[:, 0:128], dum_t[:],
                                 start=True, stop=True)

            # tb = 0: weave down-weight generation into the it-iterations.
            # Down quad q feeds the it=q slice of the down weight; the ht-loop
            # consumes them only after all 8 quads, so quads 0..5 ride stages
            # 0..5 and stage 6 carries quads 6 and 7.  B-parts reuse the A
            # banks (gen pool) after the A copies drain, so A and B are split
            # across the stage.
            int_tiles = []
            for it in range(8):
                if it < 6:
                    hooks = {3: [lambda q=it: quad_A("down", q, "gpsum")],
                             11: [lambda q=it: quad_B("down", q, "gpsum")]}
                elif it == 6:
                    hooks = {1: [lambda: quad_A("down", 6, "gpsum")],
                             5: [lambda: quad_A("down", 7, "gpsum")],
                             9: [lambda: quad_B("down", 6, "gpsum")],
                             13: [lambda: quad_B("down", 7, "gpsum")]}
                else:
                    hooks = None
                int_tiles.append(emit_it_iter(it, hs_cur, hooks))
            hs_next = emit_hs(1)
            emit_ht_loop(0, int_tiles)

            for tb in range(1, 16):
                hs_cur = hs_next
                hs_next = emit_hs(tb + 1) if tb + 1 < 16 else None
                int_tiles = [emit_it_iter(it, hs_cur) for it in range(8)]
                emit_ht_loop(tb, int_tiles)

    nc.compile()
    return nc


_NC_CACHE = None


def _get_program():
    global _NC_CACHE
    if _NC_CACHE is None:
        _NC_CACHE = _build_program()
    return _NC_CACHE


def _in_maps(hidden_states, gate_cp, up_cp, down_cp):
    hs = np.asarray(hidden_states, dtype=np.float32).reshape(NTOK, HIDDEN).T
    hs = np.ascontiguousarray(hs).astype(ml_dtypes.bfloat16)
    dum = np.zeros((128, 512), dtype=ml_dtypes.bfloat16)
    cps = {"gate": np.asarray(gate_cp, dtype=np.float32),
           "up": np.asarray(up_cp, dtype=np.float32),
           "down": np.asarray(down_cp, dtype=np.float32)}
    maps = []
    for r in range(NCORES):
        idx_gu, us_gu, mask_gu = _STATIC_GU[r]
        idx_dn, us_dn, mask_dn = _STATIC_DN[r]
        m = {"hsT": hs, "dum": dum, "va": _VA4, "vb": _VB4,
             "mask_gu": mask_gu, "mask_dn": mask_dn,
             "zp_gate": _zp4_for(cps["gate"], idx_gu, us_gu),
             "zp_up": _zp4_for(cps["up"], idx_gu, us_gu),
             "zp_down": _zp4_for(cps["down"], idx_dn, us_dn)}
        maps.append(m)
    return maps


def kernel(hidden_states, gate_cp, up_cp, down_cp, _trace=False):
    nc = _get_program()
    maps = _in_maps(hidden_states, gate_cp, up_cp, down_cp)
    res = run_bass_kernel_spmd(nc, maps, core_ids=list(range(NCORES)),
                               trace=_trace)
    out_T = np.zeros((HIDDEN, NTOK), dtype=np.float32)
    for r in range(NCORES):
        out_T += np.asarray(res.results[r]["outT"], dtype=np.float32)
    out = np.ascontiguousarray(out_T.T).reshape(4, 2048, HIDDEN)
    if _trace:
        kernel.last_results = res
    return out


# revision 23
# speedup vs baseline: 1.0194x; 1.0194x over previous
"""Trainium2 Bass kernel for nn_DeepSeekNeuralMLP (SwiGLU MLP with
Catmull-Rom-spline-reconstructed weights), tensor-parallel over 8 NeuronCores.

Strategy (Megatron-style, fused single-phase):
  - gate/up weights [8192, 2048] sharded over the intermediate dim: core r owns
    rows [r*1024, (r+1)*1024).  down weight [2048, 8192] sharded over its input
    (intermediate) dim: core r owns columns [r*1024, (r+1)*1024).  Each core
    produces a partial output [2048, 8192] in bf16; the host sums the 8
    partials in f32 and transposes to the final [4, 2048, 2048].
  - Spline reconstruction runs on-device into bf16 weight tiles held in SBUF
    for the whole kernel.  The static sampling grid factors into 128-sample
    chunks; within a chunk the control interval index takes at most two values
    (j_c, j_c+1), so each chunk is two cubic evaluations blended by a static
    mask.  Host-side input prep gathers control-point taps and scales them by
    static u-powers (z-rows); the device does two small matmuls per 512-chunk
    block (static Vandermonde-style lhsT VA/VB) and a copy + predicated-patch
    select.
  - Single fused main loop over 16 blocks of 512 tokens: gate/up matmuls ->
    SwiGLU -> down matmul straight out of SBUF (no DRAM spill of the
    intermediate).
  - HAM discipline: K=21 gen matmuls only light one array quadrant and count
    as PE-idle for the HAM clock gate, so a gen phase run in isolation sits at
    the cold 1.2 GHz clock.  Fix: full-K dummy matmuls warm the gate at t=0,
    and all weight generation (beyond the first two super-blocks) is WOVEN
    between the full-K matmuls of token-block 0, which keeps the activity
    monitor busy and hides the gen drains (scalar copy + predicated patch)
    under main-loop PE work.  gate/up use the strided chunk order so
    it-iteration i depends only on super-blocks {2i, 2i+1}.
"""
import contextlib
import numpy as np
import ml_dtypes
from math import comb

import concourse.bass as bass
from concourse import bacc, tile, mybir
from concourse.bass_utils import run_bass_kernel_spmd

# ----------------------------------------------------------------------------
# static problem geometry (hardcoded; must match the reference)
# ----------------------------------------------------------------------------
HIDDEN = 2048
INTER = 8192
NTOK = 8192                    # 4 * 2048 tokens
NCORES = 8
N = INTER * HIDDEN             # samples per weight (same for all three)
NCTRL = max(16, int(N / 128.9))
NCHUNK = N // 128
CPB = NCHUNK // NCORES         # 16384 chunks per core per weight
IC = INTER // NCORES           # 1024 intermediate per core

F32 = mybir.dt.float32
F32R = mybir.dt.float32r
BF16 = mybir.dt.bfloat16
U8 = mybir.dt.uint8

_B_COEF = 0.5 * np.array([
    [0.0, -1.0,  2.0, -1.0],
    [2.0,  0.0, -5.0,  3.0],
    [0.0,  1.0,  4.0, -3.0],
    [0.0,  0.0, -1.0,  1.0],
], dtype=np.float64)           # Catmull-Rom basis b_t(f) coeffs, [tap, power]


def _static_tables():
    t = np.linspace(0.0, NCTRL - 1.0, N, dtype=np.float64)
    i = np.clip(np.floor(t).astype(np.int64), 0, NCTRL - 2)
    k0 = np.arange(NCHUNK, dtype=np.int64) * 128
    j = i[k0]
    iv = i.reshape(NCHUNK, 128)
    m = (iv == j[:, None]).sum(axis=1)
    u = t[k0] - j
    delta = (NCTRL - 1.0) / (N - 1.0)
    return j, u, m, delta


_J, _U, _M, _DELTA = _static_tables()


def _bderiv(y):
    y = np.asarray(y, dtype=np.float64)
    out = np.zeros((4, 4) + y.shape, dtype=np.float64)
    for e in range(4):
        for tp in range(4):
            for p in range(e, 4):
                out[e, tp] += comb(p, e) * _B_COEF[tp, p] * y ** (p - e)
    return out


def _va_vb():
    """Row map: z = (e-1)*5 + tau for e in 1..3 (u-power-scaled tap rows),
    z = 15 dummy zero row, z = 16 + tau for raw (e = 0) tap rows."""
    s = np.arange(128, dtype=np.float64)
    dA = _bderiv(s * _DELTA)
    dB = _bderiv(s * _DELTA - 1.0)
    VA = np.zeros((21, 128), dtype=np.float64)
    VB = np.zeros((21, 128), dtype=np.float64)
    for e in range(4):
        for tp in range(4):
            zA = 16 + tp if e == 0 else (e - 1) * 5 + tp
            zB = 16 + (tp + 1) if e == 0 else (e - 1) * 5 + (tp + 1)
            VA[zA] = dA[e, tp]
            VB[zB] = dB[e, tp]
    return VA.astype(np.float32), VB.astype(np.float32)


_VA, _VB = _va_vb()


def _chunklists():
    """gate/up (strided order): tile column q = i_local*16 + hb; lhsT tile
    (it, kt) = cols [it*2048 + kt : +2033 : 16]; tile (it, *) only touches
    cols [it*2048, (it+1)*2048) = super-blocks {2it, 2it+1}.
    down (contiguous order): tile column q = ib*2048 + h <-> global chunk
    h*64 + r*8 + ib; lhsT tile (ht, it) = cols [it*2048 + ht*128, +128)."""
    gu = np.arange(NCHUNK, dtype=np.int64).reshape(NCORES, CPB)
    h = np.arange(HIDDEN, dtype=np.int64)
    ib = np.arange(8, dtype=np.int64)
    dn = np.empty((NCORES, CPB), dtype=np.int64)
    for r in range(NCORES):
        dn[r] = (h[None, :] * 64 + r * 8 + ib[:, None]).reshape(-1)
    return gu, dn


_CL_GU, _CL_DN = _chunklists()


def _static_for_clist(cl):
    """cp gather indices [5, CPB], u-power rows [15, CPB], mask [128, CPB]."""
    j = _J[cl]
    u = _U[cl]
    m = _M[cl]
    idx = np.clip(j[None, :] + np.arange(-1, 4)[:, None], 0, NCTRL - 1)
    us = np.zeros((15, cl.size), dtype=np.float32)
    for e in range(1, 4):
        us[(e - 1) * 5:(e - 1) * 5 + 5, :] = (u ** e).astype(np.float32)[None, :]
    s = np.arange(128, dtype=np.int64)
    mask = (s[:, None] >= m[None, :]).astype(np.uint8)
    return idx, np.ascontiguousarray(us), np.ascontiguousarray(mask)


_STATIC_GU = [_static_for_clist(_CL_GU[r]) for r in range(NCORES)]
_STATIC_DN = [_static_for_clist(_CL_DN[r]) for r in range(NCORES)]


def _zp_for(cp, idx, us):
    """z-operand [21, CPB]: rows 0..14 = taps * u^e (e=1..3), row 15 = 0,
    rows 16..20 = raw taps."""
    rows = np.take(cp, idx).astype(np.float32)
    zp = np.zeros((21, idx.shape[1]), dtype=np.float32)
    zp[0:15] = np.tile(rows, (3, 1)) * us
    zp[16:21] = rows
    return np.ascontiguousarray(zp)


def _zp4_for(cp, idx, us):
    """Row-packed z-operand [128, CPB/4] for tile_position generation: four
    consecutive 512-chunk blocks stacked on partition strips 32i+z.  Quad qb
    covers blocks 4qb..4qb+3; strip i holds block 4qb+i."""
    zp = _zp_for(cp, idx, us)
    zpv = zp.reshape(21, CPB // 2048, 4, 512)     # [z, qb, i, t]
    zp4 = np.zeros((128, CPB // 4), dtype=np.float32)
    for i in range(4):
        zp4[32 * i:32 * i + 21] = zpv[:, :, i, :].reshape(21, CPB // 4)
    return np.ascontiguousarray(zp4)


def _va4_vb4():
    va4 = np.zeros((128, 128), dtype=np.float32)
    vb4 = np.zeros((128, 128), dtype=np.float32)
    for i in range(4):
        va4[32 * i:32 * i + 21] = _VA
        vb4[32 * i:32 * i + 21] = _VB
    return va4, vb4


_VA4, _VB4 = _va4_vb4()


# ----------------------------------------------------------------------------
# device program
# ----------------------------------------------------------------------------
def _build_program():
    nc = bacc.Bacc("TRN2", target_bir_lowering=False, debug=False,
                   num_devices=NCORES)

    hsT = nc.dram_tensor("hsT", [HIDDEN, NTOK], BF16, kind="ExternalInput")
    dum_d = nc.dram_tensor("dum", [128, 512], BF16, kind="ExternalInput")
    va_d = nc.dram_tensor("va", [128, 128], F32R, kind="ExternalInput")
    vb_d = nc.dram_tensor("vb", [128, 128], F32R, kind="ExternalInput")
    zp_d = {w: nc.dram_tensor(f"zp_{w}", [128, CPB // 4], F32R,
                              kind="ExternalInput")
            for w in ("gate", "up", "down")}
    mask_gu_d = nc.dram_tensor("mask_gu", [128, CPB], U8, kind="ExternalInput")
    mask_dn_d = nc.dram_tensor("mask_dn", [128, CPB], U8, kind="ExternalInput")
    outT = nc.dram_tensor("outT", [HIDDEN, NTOK], BF16, kind="ExternalOutput")

    with tile.TileContext(nc) as tc:
        with contextlib.ExitStack() as ctx:
            pools = {
                "const": ctx.enter_context(tc.tile_pool(name="const", bufs=1)),
                "w": ctx.enter_context(tc.tile_pool(name="w", bufs=1)),
                "zp": ctx.enter_context(tc.tile_pool(name="zp", bufs=8)),
                "mask": ctx.enter_context(tc.tile_pool(name="mask", bufs=8)),
                "hs": ctx.enter_context(tc.tile_pool(name="hs", bufs=32)),
                "sil": ctx.enter_context(tc.tile_pool(name="sil", bufs=3)),
                "inter": ctx.enter_context(tc.tile_pool(name="inter", bufs=12)),
                "out": ctx.enter_context(tc.tile_pool(name="out", bufs=4)),
                "psum": ctx.enter_context(
                    tc.tile_pool(name="psum", bufs=4, space="PSUM")),
                "gpsum": ctx.enter_context(
                    tc.tile_pool(name="gpsum", bufs=4, space="PSUM")),
            }
            dum_t = pools["const"].tile([128, 512], BF16, tag="dum")
            nc.sync.dma_start(dum_t[:], dum_d[:])
            va_t = pools["const"].tile([128, 128], F32R, tag="va")
            vb_t = pools["const"].tile([128, 128], F32R, tag="vb")
            nc.sync.dma_start(va_t[:], va_d[:])
            nc.sync.dma_start(vb_t[:], vb_d[:])

            gate_t = pools["w"].tile([128, CPB], BF16, tag="gate", name="gate_t")
            up_t = pools["w"].tile([128, CPB], BF16, tag="up", name="up_t")
            down_t = pools["w"].tile([128, CPB], BF16, tag="down", name="down_t")

            # ---- weight generation: tile_position row-packed quads --------
            # Quad qb of weight w = blocks 4qb..4qb+3 (weight cols
            # [qb*2048,(qb+1)*2048)).  The four K=32 strip matmuls run
            # concurrently on distinct array row-groups, so the full 128-row
            # array is active — unlike K=21 matmuls, this counts as PE-busy
            # for the HAM clock gate.
            wmap = {}
            gen_stash = {}

            def quad_A(w, qb, pa_pool):
                wtile, zdram, mdram = wmap[w]
                zq = pools["zp"].tile([128, 512], F32R, tag="zp", name="zq")
                nc.sync.dma_start(zq[:], zdram[:, qb * 512:(qb + 1) * 512])
                pas, mts = [], []
                for i in range(4):
                    pa = pools[pa_pool].tile([128, 512], F32, tag="ps",
                                             name="pa4")
                    nc.tensor.matmul(pa[:], va_t[32 * i:32 * i + 32, :],
                                     zq[32 * i:32 * i + 32, :],
                                     start=True, stop=True,
                                     tile_position=(32 * i, 0))
                    pas.append(pa)
                for i in range(4):
                    col = (4 * qb + i) * 512
                    mt = pools["mask"].tile([128, 512], U8, tag="mask",
                                            name="mt")
                    nc.sync.dma_start(mt[:], mdram[:, col:col + 512])
                    nc.scalar.copy(wtile[:, col:col + 512], pas[i][:])
                    mts.append(mt)
                gen_stash[(w, qb)] = (zq, mts)

            def quad_B(w, qb, pb_pool):
                wtile, zdram, mdram = wmap[w]
                zq, mts = gen_stash.pop((w, qb))
                pbs = []
                for i in range(4):
                    pb = pools[pb_pool].tile([128, 512], F32, tag="ps",
                                             name="pb4")
                    nc.tensor.matmul(pb[:], vb_t[32 * i:32 * i + 32, :],
                                     zq[32 * i:32 * i + 32, :],
                                     start=True, stop=True,
                                     tile_position=(32 * i, 0))
                    pbs.append(pb)
                for i in range(4):
                    col = (4 * qb + i) * 512
                    nc.vector.copy_predicated(wtile[:, col:col + 512],
                                              mts[i][:], pbs[i][:])

            def emit_hs(tb, kts=range(16)):
                tiles = []
                for kt in kts:
                    t = pools["hs"].tile([128, 512], BF16, tag="hs", name="hst")
                    nc.sync.dma_start(
                        t[:], hsT[kt * 128:(kt + 1) * 128, tb * 512:(tb + 1) * 512])
                    tiles.append(t)
                return tiles

            def emit_it_iter(it, hs_tiles, hooks=None):
                """One intermediate block: 32 gate/up matmuls + SwiGLU.
                `hooks` maps kt -> callables emitted after that kt's matmuls
                (used to weave down-weight generation under main PE work)."""
                pg = pools["psum"].tile([128, 512], F32, tag="ps", name="pg")
                pu = pools["psum"].tile([128, 512], F32, tag="ps", name="pu")
                for kt in range(16):
                    base = it * 2048 + kt
                    lg = gate_t[:, base:base + 2033:16]
                    lu = up_t[:, base:base + 2033:16]
                    rhs = hs_tiles[kt][:]
                    nc.tensor.matmul(pg[:], lg, rhs,
                                     start=(kt == 0), stop=(kt == 15))
                    nc.tensor.matmul(pu[:], lu, rhs,
                                     start=(kt == 0), stop=(kt == 15))
                    if hooks and kt in hooks:
                        for fn in hooks[kt]:
                            fn()
                sil = pools["sil"].tile([128, 512], F32, tag="sil", name="sil")
                nc.scalar.activation(sil[:], pg[:],
                                     mybir.ActivationFunctionType.Silu)
                itile = pools["inter"].tile([128, 512], BF16, tag="itile",
                                            name="itile")
                nc.vector.tensor_mul(itile[:], sil[:], pu[:])
                return itile

            def emit_ht_loop(tb, int_tiles):
                for ht in range(16):
                    pd = pools["psum"].tile([128, 512], F32, tag="ps", name="pd")
                    for it in range(8):
                        base = it * 2048 + ht * 128
                        ld = down_t[:, base:base + 128]
                        nc.tensor.matmul(pd[:], ld, int_tiles[it][:],
                                         start=(it == 0), stop=(it == 7))
                    ot = pools["out"].tile([128, 512], BF16, tag="ot", name="ot")
                    nc.scalar.copy(ot[:], pd[:])
                    nc.sync.dma_start(
                        outT[ht * 128:(ht + 1) * 128, tb * 512:(tb + 1) * 512],
                        ot[:])

            wmap.update({
                "gate": (gate_t, zp_d["gate"], mask_gu_d),
                "up": (up_t, zp_d["up"], mask_gu_d),
                "down": (down_t, zp_d["down"], mask_dn_d),
            })

            # ---- schedule ----
            # HAM warm-up: full-K dummy matmuls (never read) flip the clock
            # gate while the first gen DMAs land.
            for _ in range(16):
                pdum = pools["gpsum"].tile([128, 512], F32, tag="ps",
                                           name="pdum")
                nc.tensor.matmul(pdum[:], dum_t[:, 0:128], dum_t[:],
                                 start=True, stop=True)

            # gate/up generation, serial, drain-bound (~44us): A-matmuls use
            # the gen PSUM pool, B-matmuls the main pool (idle until tb0).
            for qb in range(8):
                quad_A("gate", qb, "gpsum")
                quad_B("gate", qb, "psum")
                quad_A("up", qb, "gpsum")
                quad_B("up", qb, "psum")

            hs_cur = emit_hs(0)

            # re-warm right before the dense stream (the drain-bound gen
            # phase above has low PE duty and lets the clock gate re-throttle)
            for _ in range(8):
                pdum = pools["gpsum"].tile([128, 512], F32, tag="ps",
                                           name="pdum2")
                nc.tensor.matmul(pdum[:], dum_t[:, 0:128], dum_t[:],
                                 start=True, stop=True)

            # tb = 0: weave down-weight generation into the it-iterations.
            # Down quad q feeds the it=q slice of the down weight; the ht-loop
            # consumes them only after all 8 quads, so quads 0..5 ride stages
            # 0..5 and stage 6 carries quads 6 and 7.  B-parts reuse the A
            # banks (gen pool) after the A copies drain, so A and B are split
            # across the stage.
            int_tiles = []
            for it in range(8):
                if it < 6:
                    hooks = {3: [lambda q=it: quad_A("down", q, "gpsum")],
                             11: [lambda q=it: quad_B("down", q, "gpsum")]}
                elif it == 6:
                    hooks = {1: [lambda: quad_A("down", 6, "gpsum")],
                             5: [lambda: quad_A("down", 7, "gpsum")],
                             9: [lambda: quad_B("down", 6, "gpsum")],
                             13: [lambda: quad_B("down", 7, "gpsum")]}
                else:
                    hooks = None
                int_tiles.append(emit_it_iter(it, hs_cur, hooks))
            hs_next = emit_hs(1)
            emit_ht_loop(0, int_tiles)

            for tb in range(1, 16):
                hs_cur = hs_next
                hs_next = emit_hs(tb + 1) if tb + 1 < 16 else None
                int_tiles = [emit_it_iter(it, hs_cur) for it in range(8)]
                emit_ht_loop(tb, int_tiles)

    nc.compile()
    return nc


_NC_CACHE = None


def _get_program():
    global _NC_CACHE
    if _NC_CACHE is None:
        _NC_CACHE = _build_program()
    return _NC_CACHE


def _in_maps(hidden_states, gate_cp, up_cp, down_cp):
    hs = np.asarray(hidden_states, dtype=np.float32).reshape(NTOK, HIDDEN).T
    hs = np.ascontiguousarray(hs).astype(ml_dtypes.bfloat16)
    dum = np.zeros((128, 512), dtype=ml_dtypes.bfloat16)
    cps = {"gate": np.asarray(gate_cp, dtype=np.float32),
           "up": np.asarray(up_cp, dtype=np.float32),
           "down": np.asarray(down_cp, dtype=np.float32)}
    maps = []
    for r in range(NCORES):
        idx_gu, us_gu, mask_gu = _STATIC_GU[r]
        idx_dn, us_dn, mask_dn = _STATIC_DN[r]
        m = {"hsT": hs, "dum": dum, "va": _VA4, "vb": _VB4,
             "mask_gu": mask_gu, "mask_dn": mask_dn,
             "zp_gate": _zp4_for(cps["gate"], idx_gu, us_gu),
             "zp_up": _zp4_for(cps["up"], idx_gu, us_gu),
             "zp_down": _zp4_for(cps["down"], idx_dn, us_dn)}
        maps.append(m)
    return maps


def kernel(hidden_states, gate_cp, up_cp, down_cp, _trace=False):
    nc = _get_program()
    maps = _in_maps(hidden_states, gate_cp, up_cp, down_cp)
    res = run_bass_kernel_spmd(nc, maps, core_ids=list(range(NCORES)),
                               trace=_trace)
    out_T = np.zeros((HIDDEN, NTOK), dtype=np.float32)
    for r in range(NCORES):
        out_T += np.asarray(res.results[r]["outT"], dtype=np.float32)
    out = np.ascontiguousarray(out_T.T).reshape(4, 2048, HIDDEN)
    if _trace:
        kernel.last_results = res
    return out
